# revision 1
# baseline (speedup 1.0000x reference)
"""Causal self-attention (B=2, T=2048, C=768, H=12, D=64) on 8 TRN2
NeuronCores via Bass/Tile, SPMD.

Sharding: core = b * 4 + hg  (batch b in {0,1}, head-group hg in {0..3},
3 heads each).  Each core computes a partial projection output
yp[b] = attn_out(heads of hg) @ W_proj[rows of hg]; the host sums the 4
partials per batch.

All on-chip data is bf16 (PSUM accumulation fp32); the output is
returned as bf16 and upcast on the host.  Layouts (per core):
  qk0 [128, T] = [qT(h0); qT(h1)]   qk1 [128, T] = [kT(h0); kT(h1)]
  qk2 [128, T] = [qT(h2); kT(h2)]   k2a [64, T]  = kT(h2) re-based to
                                    partition 0 via SBUF->SBUF DMA so the
                                    h2 scores matmul satisfies the
                                    lhsT/rhs same-base-partition rule.
  v_big [128, 16*195]  per j-block of 128 keys: 3 heads x (64 v cols +
                       1 ones col) -> the ones column makes the softmax
                       denominator fall out of the attn@v matmul (row 64).
  S^T strips [128, 1024] PSUM, exp'd on ACT into bf16 et tiles; causal
  masking = one precomputed bf16 mask tile * DVE multiply on the two
  diagonal strips of each (head, i-chunk).
  softmax normalization: DVE reciprocal of denom row 64 -> PE
  ones-column matmul broadcasts it to 64 partitions -> DVE multiply.
  rawT01 [128, T] packs heads 0,1 (h1 slices arrive via SBUF->SBUF DMA
  partition shift); rawT2 [64, T] holds h2; the output projection then
  needs only 2 contraction tiles per 128-query block.
"""

from contextlib import ExitStack

import numpy as np

import concourse.bass as bass
import concourse.mybir as mybir
import concourse.tile as tile
from concourse import bacc

F32 = mybir.dt.float32
BF16 = mybir.dt.bfloat16

B, T, C, H, D = 2, 2048, 768, 12, 64
HL = 3            # heads per core
NCORES = 8
KC = C // 128     # 6 contraction chunks over C
NT = T // 512     # 4 i-chunks of 512 queries
TB = T // 128     # 16 j-blocks of 128 keys
VW = HL * 65      # 195 v columns per j-block (3 heads x (64 + ones))
SCALE = D ** -0.5


def build_program(reps: int = 1) -> bacc.Bacc:
    nc = bacc.Bacc("TRN2", target_bir_lowering=False, debug=False)

    xT_d = nc.dram_tensor("xT", [C, T], BF16, kind="ExternalInput").ap()
    wqk_d = nc.dram_tensor("wqk", [128, KC * 384], BF16, kind="ExternalInput").ap()
    wv_d = nc.dram_tensor("wv", [128, KC * VW], BF16, kind="ExternalInput").ap()
    wp_d = nc.dram_tensor("wp", [128, 1536], BF16, kind="ExternalInput").ap()
    yp_d = nc.dram_tensor("yp", [T, C], BF16, kind="ExternalOutput").ap()

    with tile.TileContext(nc) as tc, ExitStack() as ctx:
        per = ctx.enter_context(tc.tile_pool(name="per", bufs=1))
        etpool = ctx.enter_context(tc.tile_pool(name="et", bufs=6))
        rcpool = ctx.enter_context(tc.tile_pool(name="rc", bufs=4))
        h1pool = ctx.enter_context(tc.tile_pool(name="h1", bufs=4))
        ybpool = ctx.enter_context(tc.tile_pool(name="yb", bufs=2))
        sc_ps = ctx.enter_context(tc.tile_pool(name="scps", bufs=2, space="PSUM"))
        av_ps = ctx.enter_context(tc.tile_pool(name="avps", bufs=2, space="PSUM"))

        # ---- persistent tiles ----
        wqk = per.tile([128, KC * 384], BF16, tag="wqk", name="wqk")
        wv = per.tile([128, KC * VW], BF16, tag="wv", name="wv")
        wp = per.tile([128, 1536], BF16, tag="wp", name="wp")
        qk0 = per.tile([128, T], BF16, tag="qk0", name="qk0")
        qk1 = per.tile([128, T], BF16, tag="qk1", name="qk1")
        qk2 = per.tile([128, T], BF16, tag="qk2", name="qk2")
        k2a = per.tile([64, T], BF16, tag="k2a", name="k2a")
        v_big = per.tile([128, TB * VW], BF16, tag="vbig", name="v_big")
        rawT01 = per.tile([128, T], BF16, tag="r01", name="rawT01")
        rawT2 = per.tile([64, T], BF16, tag="r2", name="rawT2")
        mask = per.tile([128, 2048], BF16, tag="mask", name="mask")
        ones64 = per.tile([65, 64], BF16, tag="o64", name="ones64")
        onec = per.tile([128, 1], BF16, tag="onec", name="onec")

        # loop-invariant constants, built once outside the rep loop
        nc.vector.memset(ones64[:], 1.0)
        nc.vector.memset(onec[:], 1.0)
        # causal mask for the 4 diagonal 128x512 sub-blocks: keep where
        # col - 128*jj - p >= 0
        nc.vector.memset(mask[:], 1.0)
        m3 = mask.rearrange("p (j c) -> p j c", j=4)
        nc.gpsimd.affine_select(
            out=m3, in_=m3,
            compare_op=mybir.AluOpType.is_ge,
            fill=0.0, base=0,
            pattern=[[-128, 4], [1, 512]],
            channel_multiplier=-1,
        )

        def body(_iv=None):
            with tc.tile_pool(name="x", bufs=1) as xpool:
                xsb = [xpool.tile([128, T], BF16, tag=f"x{k}", name=f"x{k}")
                       for k in range(KC)]
                # x chunk 0 and wqk first: the phase-1 wave can start on
                # them while the rest of x streams in
                nc.sync.dma_start(xsb[0][:, 0:1024], xT_d[0:128, 0:1024])
                nc.sync.dma_start(wqk[:, 0:384], wqk_d[:, 0:384])
                nc.sync.dma_start(xsb[0][:, 1024:], xT_d[0:128, 1024:])
                nc.sync.dma_start(wqk[:, 384:], wqk_d[:, 384:])
                for k in range(1, KC):
                    nc.sync.dma_start(xsb[k][:], xT_d[k * 128:(k + 1) * 128, :])
                nc.sync.dma_start(wv[:], wv_d[:, :])
                nc.sync.dma_start(wp[:], wp_d[:, :])

                # ---- phase 1 qk: k-outer accumulation in waves of 6 so PE
                # starts as soon as x chunk 0 lands ----
                qkt = [qk0, qk1, qk2]
                for wave in range(2):  # t-chunks (0,1) then (2,3)
                    accs = []
                    for half in range(2):
                        s = sc_ps.tile([128, 1024], F32, tag="sc", name="sps")
                        accs += [s[:, 0:512], s[:, 512:1024]]
                    for half in range(2):
                        m = av_ps.tile([128, 512], F32, tag="av", name="bps",
                                       bufs=3)
                        accs.append(m[:, :])
                    jobs = [(2 * wave + dt_, m) for dt_ in range(2)
                            for m in range(3)]
                    for k in range(KC):
                        for a, (t_, m) in zip(accs, jobs):
                            nc.tensor.matmul(
                                a,
                                wqk[:, k * 384 + m * 128:k * 384 + (m + 1) * 128],
                                xsb[k][:, t_ * 512:(t_ + 1) * 512],
                                start=(k == 0), stop=(k == KC - 1),
                            )
                    for idx, (a, (t_, m)) in enumerate(zip(accs, jobs)):
                        if idx % 2 == 0:
                            nc.vector.tensor_copy(
                                qkt[m][:, t_ * 512:(t_ + 1) * 512], a)
                        else:
                            nc.scalar.copy(
                                qkt[m][:, t_ * 512:(t_ + 1) * 512], a)
                    for dt_ in range(2):
                        t_ = 2 * wave + dt_
                        nc.sync.dma_start(
                            k2a[:, t_ * 512:(t_ + 1) * 512],
                            qk2[64:128, t_ * 512:(t_ + 1) * 512])

                # ---- phase 1 v ----
                for tb in range(TB):
                    ps = sc_ps.tile([128, 1024], F32, tag="sc", name="sps")
                    for k in range(KC):
                        nc.tensor.matmul(
                            ps[:, 0:VW],
                            xsb[k][:, tb * 128:(tb + 1) * 128],
                            wv[:, k * VW:(k + 1) * VW],
                            start=(k == 0), stop=(k == KC - 1),
                        )
                    nc.vector.tensor_copy(
                        v_big[:, tb * VW:(tb + 1) * VW], ps[:, 0:VW])
                # ones columns (denominator trick), one strided DVE write
                ones_view = v_big.rearrange(
                    "p (t h c) -> p t h c", t=TB, h=HL)[:, :, :, 64:65]
                nc.vector.tensor_copy(
                    ones_view, onec.broadcast_to([128, TB, HL, 1]))

            # ---- phase 2 + 3, i-chunk outer so the output projection of
            # chunk ic streams out while chunk ic+1 is still attending ----
            def qT(h):
                return (qk0[0:64], qk0[64:128], qk2[0:64])[h]

            def kT(h):
                return (qk1[0:64], qk1[64:128], k2a[0:64])[h]

            # Deferred work (normalization of the previous unit, output
            # projection of the previous i-chunk) is drained one closure at a
            # time between this unit's score/attend slots, so its cross-engine
            # latency hides behind queued PE work instead of head-of-line
            # blocking the PE queue.
            deferred = []

            def pop_deferred():
                if deferred:
                    deferred.pop(0)()

            def norm_deferred(av, h, ic):
                rcp = rcpool.tile([65, 512], BF16, tag="rcp", name="rcp")
                with nc.allow_low_precision(reason="softmax denom in bf16"):
                    nc.vector.reciprocal(rcp[64:65, :], av[64:65, :])
                # DVE tensor_tensor may read only one PSUM operand, so stage
                # the unnormalized rows in SBUF (alternating engines)
                un = h1pool.tile([64, 512], BF16, tag="un", name="un")
                if ic == NT - 1:
                    nc.scalar.copy(un[:, :], av[0:64, :])
                else:
                    nc.vector.tensor_copy(un[:, :], av[0:64, :])

                def fin():
                    # broadcast into the (already-consumed) av tile: its
                    # values live on in `un`, so rows 0-63 are reusable PSUM
                    bc = av
                    nc.tensor.matmul(
                        bc[0:64, 0:512], ones64[64:65, :], rcp[64:65, :],
                        start=True, stop=True)
                    if h == 0:
                        dst = rawT01[0:64, ic * 512:(ic + 1) * 512]
                    elif h == 2:
                        dst = rawT2[0:64, ic * 512:(ic + 1) * 512]
                    else:
                        h1t = h1pool.tile([64, 512], BF16, tag="h1t", name="h1t")
                        dst = h1t[:, :]
                    nc.vector.tensor_mul(dst, un[:, :], bc[0:64, 0:512])
                    if h == 1:
                        nc.sync.dma_start(
                            rawT01[64:128, ic * 512:(ic + 1) * 512], dst)
                deferred.append(fin)

            def proj_chunk(ic):
                for t4 in range(4):
                    def fin(t4=t4):
                        tb = ic * 4 + t4
                        yt = ybpool.tile([128, C], BF16, tag="yb", name="yt")
                        if ic == NT - 1:
                            ps = sc_ps.tile([128, 1024], F32, tag="sc",
                                            name="sps")
                        else:
                            ps = av_ps.tile([128, 512], F32, tag="pj",
                                            name="pps", bufs=1)
                        for c0, cw in ((0, 512), (512, 256)):
                            po = c0 if ic == NT - 1 else 0
                            nc.tensor.matmul(
                                ps[:, po:po + cw],
                                rawT01[:, tb * 128:(tb + 1) * 128],
                                wp[:, c0:c0 + cw],
                                start=True, stop=False)
                            nc.tensor.matmul(
                                ps[:, po:po + cw],
                                rawT2[0:64, tb * 128:(tb + 1) * 128],
                                wp[0:64, 768 + c0:768 + c0 + cw],
                                start=False, stop=True)
                            # the last chunk's copies land in the tail where
                            # ACT is idle; elsewhere keep ACT free for exp
                            if ic == NT - 1:
                                nc.scalar.copy(
                                    yt[:, c0:c0 + cw], ps[:, po:po + cw])
                            else:
                                nc.vector.tensor_copy(
                                    yt[:, c0:c0 + cw], ps[:, po:po + cw])
                        nc.sync.dma_start(
                            yp_d[tb * 128:(tb + 1) * 128, :], yt[:])
                    deferred.append(fin)

            for ic in range(NT):
                nst = 2 * (ic + 1)  # strips of 2 j-blocks per head
                # One merged strip stream for all 3 heads, round-robin, with
                # 3 live av accumulators: the pipeline-fill bubble happens
                # once per chunk instead of once per head, and PE always has
                # another head's strips while ACT exps this one's.
                avs = [av_ps.tile([65, 512], F32, tag="av", name="av",
                                  bufs=3) for _ in range(HL)]
                # per-head strip order: one full strip first, then the
                # diagonal strips (longest exp+mask chain), then the rest;
                # the last strip is full-width, which the av trim relies on
                # for its accumulation stop
                if nst == 2:
                    order = [0, 1]
                else:
                    order = [0, nst - 2, nst - 1] + list(range(1, nst - 2))
                ets = {}
                first = {h: True for h in range(HL)}

                def scores(h, s, ic=ic, ets=ets):
                    sps = sc_ps.tile([128, 1024], F32, tag="sc", name="sps")
                    for jj in range(2):
                        jb = 2 * s + jj
                        # diagonal block: skip the fully-masked i-prefix
                        trim = (jb - 4 * ic) * 128 if jb >= 4 * ic else 0
                        nc.tensor.matmul(
                            sps[:, jj * 512 + trim:(jj + 1) * 512],
                            kT(h)[:, jb * 128:(jb + 1) * 128],
                            qT(h)[:, ic * 512 + trim:(ic + 1) * 512],
                            start=True, stop=True,
                        )
                    et = etpool.tile([128, 1024], BF16, tag="et", name="et")
                    diag = s >= 2 * ic
                    nc.scalar.activation(
                        et[:], sps[:], mybir.ActivationFunctionType.Exp,
                        scale=SCALE)
                    if diag:
                        # only the prefix+triangle region of each jj-block
                        # needs masking; beyond it everything is valid
                        mo = (s - 2 * ic) * 1024
                        d0 = 2 * (s - 2 * ic) * 128
                        for jj in range(2):
                            w = d0 + (jj + 1) * 128
                            nc.vector.tensor_mul(
                                et[:, jj * 512:jj * 512 + w],
                                et[:, jj * 512:jj * 512 + w],
                                mask[:, mo + jj * 512:mo + jj * 512 + w])
                    ets[(h, s)] = et

                def attend(h, s, ic=ic, ets=ets):
                    av = avs[h]
                    for jj in range(2):
                        jb = 2 * s + jj
                        stop = s == order[-1] and jj == 1
                        # masked diag prefix contributes zeros: skip it,
                        # except on the stop matmul (must close the full
                        # region) and the start matmul
                        trim = (jb - 4 * ic) * 128 if jb >= 4 * ic else 0
                        if stop or (first[h] and jj == 0):
                            trim = 0
                        nc.tensor.matmul(
                            av[:, trim:512],
                            v_big[:, jb * VW + h * 65:jb * VW + (h + 1) * 65],
                            ets[(h, s)][:, jj * 512 + trim:(jj + 1) * 512],
                            start=(first[h] and jj == 0),
                            stop=stop,
                        )
                    first[h] = False

                slots = [(h, s) for s in order for h in (1, 0, 2)]
                lag = 3
                done = 0

                def after_attend(hp, sp):
                    nonlocal done
                    if sp == order[-1]:
                        norm_deferred(avs[hp], hp, ic)
                        done += 1
                        if done == HL:
                            proj_chunk(ic)
                    pop_deferred()

                for i, (h, s) in enumerate(slots):
                    scores(h, s)
                    if i < 2:
                        pop_deferred()
                    if i >= lag:
                        hp, sp = slots[i - lag]
                        attend(hp, sp)
                        after_attend(hp, sp)
                for i in range(len(slots) - lag, len(slots)):
                    hp, sp = slots[i]
                    attend(hp, sp)
                    after_attend(hp, sp)
            while deferred:
                pop_deferred()

        if reps == 1:
            body()
        else:
            with tc.For_i(0, reps, 1) as iv:
                body(iv)

    nc.compile()
    return nc


# ---------------- host side ----------------

def _bf16(a):
    import ml_dtypes
    return np.asarray(a, dtype=ml_dtypes.bfloat16)


def shard_inputs(x, W_qkv, W_proj):
    def kpack(w):  # [768, n] -> [128, 6*n] with k-chunk-major columns
        n = w.shape[1]
        return np.ascontiguousarray(
            w.reshape(KC, 128, n).transpose(1, 0, 2).reshape(128, KC * n))

    in_maps = []
    for core in range(NCORES):
        b, hg = divmod(core, 4)
        xT = np.ascontiguousarray(x[b].T)                      # [C, T]
        h0 = hg * HL
        q = W_qkv[:, h0 * 64:(h0 + HL) * 64]
        k = W_qkv[:, C + h0 * 64:C + (h0 + HL) * 64]
        v = W_qkv[:, 2 * C + h0 * 64:2 * C + (h0 + HL) * 64]
        wqk = np.concatenate(
            [q[:, 0:128], k[:, 0:128], q[:, 128:192], k[:, 128:192]], axis=1)
        wv = np.zeros((C, VW), np.float32)
        for h in range(HL):
            wv[:, h * 65:h * 65 + 64] = v[:, h * 64:(h + 1) * 64]
        wp = np.zeros((128, 1536), np.float32)
        wp[:, 0:768] = W_proj[h0 * 64:(h0 + 2) * 64, :]
        wp[0:64, 768:1536] = W_proj[(h0 + 2) * 64:(h0 + 3) * 64, :]
        in_maps.append({"xT": _bf16(xT), "wqk": _bf16(kpack(wqk)),
                        "wv": _bf16(kpack(wv)), "wp": _bf16(wp)})
    return in_maps


def unshard(results):
    y = np.zeros((B, T, C), np.float64)
    for core in range(NCORES):
        b = core // 4
        y[b] += results[core]["yp"].astype(np.float64)
    return y.astype(np.float32)


# ---------------- PJRT runner (axon-tunneled NeuronCores) ----------------

_RUNNERS = {}


def get_runner(reps: int = 1):
    """Build (once) and return fn(in_maps) -> list[dict] for 8 cores."""
    if reps in _RUNNERS:
        return _RUNNERS[reps]

    import jax
    from jax.sharding import Mesh, PartitionSpec, NamedSharding
    from jax.experimental.shard_map import shard_map
    from concourse.bass2jax import (
        _bass_exec_p, install_neuronx_cc_hook, partition_id_tensor)

    nc = build_program(reps=reps)
    install_neuronx_cc_hook()

    partition_name = nc.partition_id_tensor.name if nc.partition_id_tensor else None
    in_names, out_names, out_avals = [], [], []
    for alloc in nc.m.functions[0].allocations:
        if not isinstance(alloc, mybir.MemoryLocationSet):
            continue
        name = alloc.memorylocations[0].name
        if alloc.kind == "ExternalInput":
            if name != partition_name:
                in_names.append(name)
        elif alloc.kind == "ExternalOutput":
            out_names.append(name)
            out_avals.append(jax.core.ShapedArray(
                tuple(alloc.tensor_shape), mybir.dt.np(alloc.dtype)))
    n_params = len(in_names)
    all_in_names = in_names + out_names + ([partition_name] if partition_name else [])

    def _body(*args):
        operands = list(args)
        if partition_name is not None:
            operands.append(partition_id_tensor())
        outs = _bass_exec_p.bind(
            *operands, out_avals=tuple(out_avals), in_names=tuple(all_in_names),
            out_names=tuple(out_names), lowering_input_output_aliases=(),
            sim_require_finite=True, sim_require_nnan=True, nc=nc)
        return tuple(outs)

    devices = jax.devices()[:NCORES]
    mesh = Mesh(np.asarray(devices), ("core",))
    spec = (PartitionSpec("core"),)
    fn = jax.jit(
        shard_map(_body, mesh=mesh, in_specs=spec * (n_params + len(out_names)),
                  out_specs=spec * len(out_names), check_rep=False),
        keep_unused=True)
    sharding = NamedSharding(mesh, PartitionSpec("core"))

    def run(in_maps, in_dev=None):
        if in_dev is None:
            in_dev = put_inputs(in_maps, sharding, in_names)
        zeros = [
            jax.device_put(
                np.zeros((NCORES * a.shape[0], *a.shape[1:]), a.dtype), sharding)
            for a in out_avals]
        outs = fn(*in_dev, *zeros)
        return [
            {name: np.asarray(outs[i]).reshape(NCORES, *out_avals[i].shape)[c]
             for i, name in enumerate(out_names)}
            for c in range(NCORES)]

    def put_inputs(in_maps, sharding_=None, names=None):
        import jax as _jax
        sh = sharding_ or sharding
        nm = names or in_names
        return [
            _jax.device_put(
                np.concatenate([in_maps[c][n] for c in range(NCORES)], axis=0), sh)
            for n in nm]

    run.put_inputs = put_inputs
    run.sharding = sharding
    run.in_names = in_names
    run.fn = fn
    run.out_avals = out_avals
    _RUNNERS[reps] = run
    return run


def kernel(x, W_qkv, W_proj):
    x = np.asarray(x, dtype=np.float32)
    W_qkv = np.asarray(W_qkv, dtype=np.float32)
    W_proj = np.asarray(W_proj, dtype=np.float32)
    run = get_runner(reps=1)
    results = run(shard_inputs(x, W_qkv, W_proj))
    return unshard(results)

